# revision 31
# baseline (speedup 1.0000x reference)
"""AttentionPairBias Trainium2 Bass kernel — 8-core SPMD, block-sharded.

Sharding: 128 attention blocks -> 16 blocks (512 query rows) per core, with a
64-row halo on a/s so k/v windows need no cross-core exchange.

z path runs z-STATIONARY: per (group, block, query) the [cz, 128k] z tile is
loaded as PE weights and a 17-col wz matrix (16 heads + ones column for the
mean) streams through, so the pair-bias lands directly on k-partitions — no
[18 x N] evictions and no bias transposes. A second 1-col matmul over z^2
yields the variance. Scores are computed TRANSPOSED (k x q) so the exp output
is already the AV stationary; the softmax denominator rides the AV matmul as
a ones-column appended to each v window. LN stats ride the matmuls; big
projections run fp16 with fp32 accumulation.
"""
import math
import os
import sys
from contextlib import ExitStack

import numpy as np

sys.path.insert(0, "/opt/trn_rl_repo")
sys.path.insert(0, "/opt/trn_rl_repo/concourse")

import concourse.bass as bass
import concourse.mybir as mybir
import concourse.tile as tile
from concourse import bacc, bass_utils
from concourse.masks import make_identity

B, N, CA, CS, CZ, H = 1, 4096, 768, 384, 128, 16
NQ, NK = 32, 128
DH = CA // H            # 48
NB = N // NQ            # 128
OFF = (NK - NQ) // 2    # 48
NCORE = 8
BPC = NB // NCORE       # 16 blocks per core
ROWS = BPC * NQ         # 512 own rows
HALO = 64
R = ROWS + 2 * HALO     # 640 rows incl. halo
NGRP = 4                # 4-block groups per core
EPS = 1e-5
ISCALE = 1.0 / math.sqrt(DH)
KA = CA // 128          # 6
KS = CS // 128          # 3
NT = R // 128           # 5
VW = DH + 1             # 49: v window cols per head incl. ones column

FP32 = mybir.dt.float32
FP16 = mybir.dt.float16
AF = mybir.ActivationFunctionType
ALU = mybir.AluOpType


def bcast_ap(dram, parts, n):
    """DRAM [n] -> AP [[0,parts],[1,n]] (partition broadcast)."""
    a = dram[:]
    return bass.AP(tensor=a.tensor, offset=a.offset, ap=[[0, parts], [1, n]])


def build_core_kernel():
    nc = bacc.Bacc(None, target_bir_lowering=False)

    d_zT5 = nc.dram_tensor("zT5", [NGRP, 4, CZ, NQ * NK], FP16, kind="ExternalInput")
    d_a = nc.dram_tensor("a_h", [R, CA], FP16, kind="ExternalInput")
    d_s = nc.dram_tensor("s_h", [R, CS], FP16, kind="ExternalInput")
    d_wq = nc.dram_tensor("wq", [CA, CA], FP16, kind="ExternalInput")
    d_wk = nc.dram_tensor("wk", [CA, CA], FP16, kind="ExternalInput")
    d_wv = nc.dram_tensor("wv", [CA, CA], FP16, kind="ExternalInput")
    d_wg = nc.dram_tensor("wg", [CA, CA], FP16, kind="ExternalInput")
    d_wo = nc.dram_tensor("wo", [CA, CA], FP16, kind="ExternalInput")
    d_agw = nc.dram_tensor("adaln_g_w", [CS, CA], FP16, kind="ExternalInput")
    d_asw = nc.dram_tensor("adaln_s_w", [CS, CA], FP16, kind="ExternalInput")
    d_wl = nc.dram_tensor("w_last", [CS, CA], FP16, kind="ExternalInput")
    d_agb = nc.dram_tensor("adaln_g_b", [128, KA], FP32, kind="ExternalInput")
    d_bo = nc.dram_tensor("bo_b", [128, KA], FP32, kind="ExternalInput")
    d_bl = nc.dram_tensor("b_last_b", [128, KA], FP32, kind="ExternalInput")
    d_bgf = nc.dram_tensor("bg_full", [CA], FP32, kind="ExternalInput")
    d_wzA = nc.dram_tensor("wzA17", [CZ, 17], FP16, kind="ExternalInput")
    d_csI = nc.dram_tensor("csI", [128, H, 128], FP16, kind="ExternalInput")
    d_mask = nc.dram_tensor("maskT", [128, NGRP, 128], FP16, kind="ExternalInput")
    d_cb = nc.dram_tensor("cb", [H], FP32, kind="ExternalInput")
    d_out = nc.dram_tensor("outT", [CA, ROWS], FP32, kind="ExternalOutput")

    with tile.TileContext(nc) as tc, ExitStack() as ctx:
        const = ctx.enter_context(tc.tile_pool(name="const", bufs=1))
        pers = ctx.enter_context(tc.tile_pool(name="pers", bufs=1))
        ln_p = ctx.enter_context(tc.tile_pool(name="ln", bufs=2))
        at_p = ctx.enter_context(tc.tile_pool(name="attn", bufs=4))
        psA = ctx.enter_context(tc.tile_pool(name="psA", bufs=4, space="PSUM"))
        psB = ctx.enter_context(tc.tile_pool(name="psB", bufs=3, space="PSUM"))
        psZ = ctx.enter_context(tc.tile_pool(name="psZ", bufs=1, space="PSUM"))

        def pA(shape):
            return psA.tile(shape, FP32, tag="A", name="pA")

        def pB(shape, dt=FP32):
            return psB.tile(shape, dt, tag="B", name="pB")

        # ---------- constants ----------
        I16 = const.tile([128, 128], FP16, tag="I16")
        make_identity(nc, I16)
        csI = const.tile([128, H, 128], FP16, tag="csI")
        nc.gpsimd.dma_start(out=csI, in_=d_csI[:])
        maskT = const.tile([128, NGRP, 128], FP16, tag="maskT")
        nc.gpsimd.dma_start(out=maskT, in_=d_mask[:])
        wzA = const.tile([CZ, 17], FP16, tag="wzA")
        nc.gpsimd.dma_start(out=wzA, in_=d_wzA[:])
        agb = const.tile([128, KA], FP32, tag="agb")
        nc.gpsimd.dma_start(out=agb, in_=d_agb[:])
        bob = const.tile([128, KA], FP32, tag="bob")
        nc.gpsimd.dma_start(out=bob, in_=d_bo[:])
        blb = const.tile([128, KA], FP32, tag="blb")
        nc.gpsimd.dma_start(out=blb, in_=d_bl[:])
        bg_bc = const.tile([128, CA], FP32, tag="bg_bc")
        nc.gpsimd.dma_start(out=bg_bc, in_=bcast_ap(d_bgf, 128, CA))
        cb_sb = const.tile([128, H], FP32, tag="cb_sb")
        nc.gpsimd.dma_start(out=cb_sb, in_=bcast_ap(d_cb, 128, H))
        epsb = const.tile([128, 1], FP32, tag="epsb")
        nc.vector.memset(epsb, EPS)

        def load_w(pool, dram, kt, name):
            t = pool.tile([128, kt, dram.shape[1]], FP16, tag=name)
            nc.gpsimd.dma_start(out=t, in_=dram[:].rearrange("(k p) c -> p k c", p=128))
            return t

        # ---------- persistent activations ----------
        sT = pers.tile([128, KS, ROWS], FP16, tag="sT")
        qT_hd = pers.tile([128, H // 2, ROWS], FP16, tag="qT_hd")
        kT_hd = pers.tile([128, H // 2, R], FP16, tag="kT_hd")
        vw = pers.tile([128, BPC, H * VW], FP16, tag="vw")
        g_rm = pers.tile([128, NGRP, CA], FP16, tag="g_rm")
        go_rm = pers.tile([128, NGRP, CA], FP16, tag="go_rm")
        goT = pers.tile([128, KA, ROWS], FP16, tag="goT")
        thall_a = pers.tile([128, NGRP, H, 128], FP16, tag="thall_a")
        gs_all = pers.tile([128, KA, ROWS], FP16, tag="gs_all")
        mur_a = pers.tile([128, NGRP, 128], FP16, tag="mur_a")
        mid_cm = tc.tile_pool(name="mid", bufs=1)
        mid = mid_cm.__enter__()
        a2T = mid.tile([128, KA, R], FP16, tag="a2T")

        # ========== Phase 0: z-stationary bias pipeline (emitted interleaved
        # with phase 1 — depends only on the z input, so it overlaps the LN
        # and projection phases; DMAs ride the idle Sync queue) ==========
        zb_cm = [
            tc.tile_pool(name="zp", bufs=2),
            tc.tile_pool(name="bp", bufs=1),
            tc.tile_pool(name="sp", bufs=1),
        ]
        zp, bp, sp = [cm.__enter__() for cm in zb_cm]
        _zstate = {}

        def z_chunk(g, n):
            braw = _zstate.get(g)
            if braw is None:
                braw = bp.tile([128, 4, NQ, 18], FP16, tag="braw")
                _zstate[g] = braw
            zt = zp.tile([128, NQ * NK], FP16, tag="zt")
            nc.sync.dma_start(out=zt, in_=d_zT5[g, n])
            idx = 4 * g + n
            for qh in range(2):
                ztsq = zp.tile([128, NQ * NK // 2], FP16, tag="ztsq")
                zh = zt[:, qh * 2048 : (qh + 1) * 2048]
                hi = (2 * idx + qh) % 8
                if hi < 2:
                    nc.vector.tensor_mul(ztsq, zh, zh)
                elif hi < 6:
                    nc.scalar.square(out=ztsq, in_=zh)
                else:
                    nc.gpsimd.tensor_mul(ztsq, zh, zh)
                PZ = psZ.tile([128, 16, 18], FP32, tag="Z", name="pZ")
                for qq in range(16):
                    q = qh * 16 + qq
                    nc.tensor.matmul(
                        PZ[:, qq, 0:17],
                        zt[:, q * NK : (q + 1) * NK],
                        wzA, start=True, stop=True,
                    )
                    nc.tensor.matmul(
                        PZ[:, qq, 17:18],
                        ztsq[:, qq * NK : (qq + 1) * NK],
                        wzA[:, 16:17], start=True, stop=True,
                    )
                dst = braw[:, n, qh * 16 : (qh + 1) * 16, :]
                if qh == 0:
                    nc.scalar.activation(out=dst, in_=PZ, func=AF.Copy)
                else:
                    nc.vector.tensor_copy(out=dst, in_=PZ)

        def z_stats(g):
            braw = _zstate.pop(g)
            st = sp.tile([128, 2, 4, NQ], FP32, tag="st")
            nc.scalar.mul(out=st[:, 0], in_=braw[:, :, :, 16], mul=1.0 / CZ)
            nc.scalar.mul(out=st[:, 1], in_=braw[:, :, :, 17], mul=1.0 / CZ)
            var = sp.tile([128, 4, NQ], FP32, tag="var")
            nc.vector.tensor_mul(var, st[:, 0], st[:, 0])
            nc.vector.tensor_sub(var, st[:, 1], var)
            sd = sp.tile([128, 4, NQ], FP32, tag="sd")
            nc.scalar.activation(out=sd, in_=var, func=AF.Sqrt, bias=epsb)
            rstd = sp.tile([128, 4, NQ], FP32, tag="rstd")
            nc.vector.reciprocal(out=rstd, in_=sd)
            murv = mur_a[:, g, :].rearrange("p (n q) -> p n q", n=4)
            nc.vector.tensor_mul(murv, st[:, 0], rstd)
            in0 = braw[:, :, :, 0:16].transpose([0, 3, 1, 2])
            in1 = rstd[:].unsqueeze(1).broadcast_to([128, H, 4, NQ])
            outv = thall_a[:, g].rearrange("p h (n q) -> p h n q", n=4)
            nc.vector.tensor_mul(outv, in0, in1)
            if g in (0, NGRP - 1):
                mbc = maskT[:, g, :].unsqueeze(1).broadcast_to([128, H, 128])
                nc.vector.tensor_add(thall_a[:, g], thall_a[:, g], mbc)

        z_units = []
        for g in range(NGRP):
            for n in range(4):
                z_units.append(lambda g=g, n=n: z_chunk(g, n))
            z_units.append(lambda g=g: z_stats(g))
        _zit = iter(z_units)

        def z_step(k=1):
            for _ in range(k):
                u = next(_zit, None)
                if u is not None:
                    u()

        # ================= Phase 1: LN + adaln -> a2T =================
        with tc.tile_pool(name="ph1", bufs=1) as ph1:
            w_ag = load_w(ph1, d_agw, KS, "w_ag")
            w_as = load_w(ph1, d_asw, KS, "w_as")
            s_lnT = ph1.tile([128, KS, R], FP16, tag="s_lnT")
            a_lnT = ph1.tile([128, KA, R], FP16, tag="a_lnT")

            def layernorm_rowtile(dst_T, src_dram, width, t, kt):
                x = ln_p.tile([128, width], FP16, tag=f"ln_x{width}")
                nc.gpsimd.dma_start(out=x, in_=src_dram[t * 128 : (t + 1) * 128, :])
                sub = math.gcd(512, width)
                nsub = width // sub
                st = ln_p.tile([128, nsub, 6], FP32, tag=f"ln_st{width}")
                for j in range(nsub):
                    nc.vector.bn_stats(out=st[:, j, :], in_=x[:, j * sub : (j + 1) * sub])
                mv = ln_p.tile([128, 2], FP32, tag=f"ln_mv{width}")
                nc.vector.bn_aggr(out=mv, in_=st)
                sd = ln_p.tile([128, 1], FP32, tag=f"ln_sd{width}")
                nc.scalar.activation(out=sd, in_=mv[:, 1:2], func=AF.Sqrt, bias=epsb)
                nc.vector.reciprocal(out=sd, in_=sd)
                x16 = ln_p.tile([128, width], FP16, tag=f"ln_x16{width}")
                nc.vector.tensor_scalar(
                    out=x16, in0=x, scalar1=mv[:, 0:1], scalar2=sd,
                    op0=ALU.subtract, op1=ALU.mult,
                )
                for k in range(kt):
                    pt = pB([128, 128], FP16)
                    nc.tensor.transpose(pt, x16[:, k * 128 : (k + 1) * 128], I16)
                    if k % 2 == 0:
                        nc.vector.tensor_copy(
                            out=dst_T[:, k, t * 128 : (t + 1) * 128], in_=pt
                        )
                    else:
                        nc.scalar.activation(
                            out=dst_T[:, k, t * 128 : (t + 1) * 128], in_=pt,
                            func=AF.Copy,
                        )

            for t in range(NT):
                layernorm_rowtile(s_lnT, d_s, CS, t, KS)
                z_step()
                layernorm_rowtile(a_lnT, d_a, CA, t, KA)
                z_step()

            # raw s transposed (own rows) for the final gate
            for t in range(ROWS // 128):
                x = ln_p.tile([128, CS], FP16, tag="s_raw")
                nc.gpsimd.dma_start(
                    out=x, in_=d_s[HALO + t * 128 : HALO + (t + 1) * 128, :]
                )
                for k in range(KS):
                    pt = pB([128, 128], FP16)
                    nc.tensor.transpose(pt, x[:, k * 128 : (k + 1) * 128], I16)
                    nc.vector.tensor_copy(out=sT[:, k, t * 128 : (t + 1) * 128], in_=pt)
                z_step()

            RC = 320
            for co in range(KA):
                for rc in range(R // RC):
                    z_step()
                    rs = slice(rc * RC, (rc + 1) * RC)
                    pg = pA([128, RC])
                    pv = pA([128, RC])
                    for k in range(KS):
                        nc.tensor.matmul(
                            pg, (w_ag[:, k, co * 128 : (co + 1) * 128]),
                            (s_lnT[:, k, rs]), start=(k == 0), stop=(k == KS - 1),
                        )
                    for k in range(KS):
                        nc.tensor.matmul(
                            pv, (w_as[:, k, co * 128 : (co + 1) * 128]),
                            (s_lnT[:, k, rs]), start=(k == 0), stop=(k == KS - 1),
                        )
                    gate = ln_p.tile([128, RC], FP32, tag="gate")
                    nc.scalar.activation(
                        out=gate, in_=pg, func=AF.Sigmoid, bias=agb[:, co : co + 1]
                    )
                    nc.vector.tensor_mul(gate, gate, a_lnT[:, co, rs])
                    nc.vector.tensor_add(a2T[:, co, rs], gate, pv)

        # drain any remaining z-bias work, then free its pools
        z_step(32)
        for cm in reversed(zb_cm):
            cm.__exit__(None, None, None)

        # ================= Phase 2a: q/k projections + head remap =================
        with tc.tile_pool(name="ph2a", bufs=1) as ph2a:
            w_q = load_w(ph2a, d_wq, KA, "w_q")
            w_k = load_w(ph2a, d_wk, KA, "w_k")
            qT_ct = ph2a.tile([128, KA, ROWS], FP16, tag="qT_ct")
            kT_ct = ph2a.tile([128, KA, R], FP16, tag="kT_ct")
            for co in range(KA):
                for rc in range(2):
                    rs = slice(HALO + rc * 256, HALO + (rc + 1) * 256)
                    pq = pA([128, 256])
                    for k in range(KA):
                        nc.tensor.matmul(
                            pq, (w_q[:, k, co * 128 : (co + 1) * 128]),
                            (a2T[:, k, rs]), start=(k == 0), stop=(k == KA - 1),
                        )
                    nc.vector.tensor_scalar(
                        out=qT_ct[:, co, rc * 256 : (rc + 1) * 256], in0=pq,
                        scalar1=ISCALE, scalar2=None, op0=ALU.mult,
                    )
                for rc in range(2):
                    rs = slice(rc * 320, (rc + 1) * 320)
                    pk = pA([128, 320])
                    for k in range(KA):
                        nc.tensor.matmul(
                            pk, (w_k[:, k, co * 128 : (co + 1) * 128]),
                            (a2T[:, k, rs]), start=(k == 0), stop=(k == KA - 1),
                        )
                    nc.scalar.activation(out=kT_ct[:, co, rs], in_=pk, func=AF.Copy)
            # head-aligned remap (2 heads per tile, 64-padded)
            for h in range(H):
                hp, o64 = h // 2, (h % 2) * 64
                c0 = h * DH
                while c0 < (h + 1) * DH:
                    ct, cp = c0 // 128, c0 % 128
                    ln_ = min((h + 1) * DH - c0, 128 - cp)
                    dsl = slice(o64 + (c0 - h * DH), o64 + (c0 - h * DH) + ln_)
                    nc.sync.dma_start(
                        out=qT_hd[dsl, hp, :], in_=qT_ct[cp : cp + ln_, ct, :]
                    )
                    nc.sync.dma_start(
                        out=kT_hd[dsl, hp, :], in_=kT_ct[cp : cp + ln_, ct, :]
                    )
                    c0 += ln_

        # ================= Phase 2b: v (row-major) + g =================
        with tc.tile_pool(name="ph2b", bufs=1) as ph2b:
            w_v = load_w(ph2b, d_wv, KA, "w_v")
            w_g = load_w(ph2b, d_wg, KA, "w_g")
            v_rm = ph2b.tile([128, NT, H * VW], FP16, tag="v_rm")
            ones_v = v_rm[:].rearrange("p t (h c) -> p t h c", h=H)[:, :, :, DH : DH + 1]
            nc.vector.memset(ones_v, 1.0)

            def vw_window(n):
                wlo = 16 + 32 * n
                t0, p0 = wlo // 128, wlo % 128
                n0 = 128 - p0
                nc.gpsimd.dma_start(out=vw[0:n0, n, :], in_=v_rm[p0 : p0 + n0, t0, :])
                if n0 < 128:
                    nc.gpsimd.dma_start(
                        out=vw[n0:128, n, :], in_=v_rm[0 : 128 - n0, t0 + 1, :]
                    )

            for rt in range(NT):
                for c2 in range(2):
                    pv = pA([128, 384])
                    for k in range(KA):
                        nc.tensor.matmul(
                            pv, (a2T[:, k, rt * 128 : (rt + 1) * 128]),
                            (w_v[:, k, c2 * 384 : (c2 + 1) * 384]),
                            start=(k == 0), stop=(k == KA - 1),
                        )
                    dstv = v_rm[:, rt, :].rearrange("p (h c) -> p h c", h=H)[
                        :, c2 * 8 : (c2 + 1) * 8, 0:DH
                    ]
                    nc.vector.tensor_copy(out=dstv, in_=pv)
                if rt >= 1:
                    for n in range(4 * (rt - 1), 4 * rt):
                        vw_window(n)
            for rt in range(ROWS // 128):
                for c2 in range(2):
                    pg = pA([128, 384])
                    for k in range(KA):
                        nc.tensor.matmul(
                            pg, (a2T[:, k, HALO + rt * 128 : HALO + (rt + 1) * 128]),
                            (w_g[:, k, c2 * 384 : (c2 + 1) * 384]),
                            start=(k == 0), stop=(k == KA - 1),
                        )
                    nc.vector.tensor_add(pg, pg, bg_bc[:, c2 * 384 : (c2 + 1) * 384])
                    nc.scalar.activation(
                        out=g_rm[:, rt, c2 * 384 : (c2 + 1) * 384], in_=pg,
                        func=AF.Sigmoid,
                    )
            # remaining key/value windows
            for n in range(4 * (NT - 1), BPC):
                vw_window(n)

        mid_cm.__exit__(None, None, None)
        # s-conditioned output gate, precomputed so the attention phase's
        # scalar queue runs Exp only (no activation-table thrash)
        with tc.tile_pool(name="ph7a", bufs=1) as ph7a:
            w_l = load_w(ph7a, d_wl, KS, "w_l")
            for co in range(KA):
                pl = pA([128, ROWS])
                for k in range(KS):
                    nc.tensor.matmul(
                        pl, (w_l[:, k, co * 128 : (co + 1) * 128]),
                        (sT[:, k, :]), start=(k == 0), stop=(k == KS - 1),
                    )
                nc.scalar.activation(
                    out=gs_all[:, co, :], in_=pl, func=AF.Sigmoid,
                    bias=blb[:, co : co + 1],
                )
        # ============ Phase 3-5: transposed attention (bias precomputed) ============
        # software-pipelined by 2: scores/exp for (g,h) are emitted two
        # iterations before the AV/normalize stage that consumes them, so the
        # PE never stalls head-of-line on the scalar exp.
        def att_scores(g, h):
            hp, o64 = h // 2, (h % 2) * 64
            S = pB([128, NK])
            nc.tensor.matmul(S, I16, thall_a[:, g, h, :], start=True, stop=False)
            nc.tensor.matmul(S, csI[:, h, :], mur_a[:, g, :], start=False, stop=False)
            for nn in range(4):
                n = g * 4 + nn
                wlo = 16 + 32 * n
                nc.tensor.matmul(
                    S[:, nn * NQ : (nn + 1) * NQ],
                    kT_hd[o64 : o64 + DH, hp, wlo : wlo + NK],
                    qT_hd[o64 : o64 + DH, hp, n * NQ : (n + 1) * NQ],
                    start=False, stop=(nn == 3), skip_group_check=True,
                )
            p16 = at_p.tile([128, NK], FP16, tag="p16")
            nc.scalar.activation(out=p16, in_=S, func=AF.Exp, bias=cb_sb[:, h : h + 1])
            return p16

        def att_out(g, h, p16):
            OV = pB([128, VW])
            for nn in range(4):
                n = g * 4 + nn
                nc.tensor.matmul(
                    OV[nn * NQ : (nn + 1) * NQ, :],
                    p16[:, nn * NQ : (nn + 1) * NQ],
                    vw[:, n, h * VW : (h + 1) * VW],
                    start=True, stop=True,
                    tile_position=(0, nn * NQ),
                )
            rec = at_p.tile([128, 1], FP32, tag="rec")
            nc.vector.reciprocal(out=rec, in_=OV[:, DH : DH + 1])
            nc.vector.scalar_tensor_tensor(
                out=go_rm[:, g, h * DH : (h + 1) * DH], in0=OV[:, 0:DH],
                scalar=rec, in1=g_rm[:, g, h * DH : (h + 1) * DH],
                op0=ALU.mult, op1=ALU.mult,
            )

        pipe = []
        for g in range(NGRP):
            for h in range(H):
                pipe.append((g, h, att_scores(g, h)))
                if len(pipe) > 2:
                    pg_, ph_, pp_ = pipe.pop(0)
                    att_out(pg_, ph_, pp_)
            # (g*o) transpose for this group (lags the pipeline by <=3 heads)
            if g > 0:
                for k in range(KA):
                    pt = pB([128, 128], FP16)
                    nc.tensor.transpose(
                        pt, go_rm[:, g - 1, k * 128 : (k + 1) * 128], I16
                    )
                    nc.vector.tensor_copy(
                        out=goT[:, k, (g - 1) * 128 : g * 128], in_=pt
                    )
        for pg_, ph_, pp_ in pipe:
            att_out(pg_, ph_, pp_)
        for k in range(KA):
            pt = pB([128, 128], FP16)
            nc.tensor.transpose(
                pt, go_rm[:, NGRP - 1, k * 128 : (k + 1) * 128], I16
            )
            nc.vector.tensor_copy(
                out=goT[:, k, (NGRP - 1) * 128 : NGRP * 128], in_=pt
            )

        # ================= Phase 7: output projection + gates =================
        with tc.tile_pool(name="ph7", bufs=1) as ph7:
            w_o = load_w(ph7, d_wo, KA, "w_o")
            for gh in range(2):
                rs = slice(gh * 256, (gh + 1) * 256)
                for co in range(KA):
                    po = pA([128, 256])
                    for k in range(KA):
                        nc.tensor.matmul(
                            po, (w_o[:, k, co * 128 : (co + 1) * 128]),
                            (goT[:, k, rs]), start=(k == 0), stop=(k == KA - 1),
                        )
                    o2 = at_p.tile([128, 256], FP32, tag="o2")
                    nc.vector.tensor_scalar(
                        out=o2, in0=po, scalar1=bob[:, co : co + 1], scalar2=None,
                        op0=ALU.add,
                    )
                    fin = at_p.tile([128, 256], FP32, tag="fin")
                    nc.vector.tensor_mul(fin, gs_all[:, co, rs], o2)
                    nc.gpsimd.dma_start(out=d_out[co * 128 : (co + 1) * 128, rs], in_=fin)

    nc.compile()
    return nc


def host_prep(inputs):
    a = np.ascontiguousarray(np.asarray(inputs["a"], np.float32)[0])
    s = np.ascontiguousarray(np.asarray(inputs["s"], np.float32)[0])
    z = np.asarray(inputs["z"], np.float32)[0]
    gz = np.asarray(inputs["gz"], np.float32)
    bz = np.asarray(inputs["bz"], np.float32)
    wz = np.asarray(inputs["wz"], np.float32)
    wz2 = gz[:, None] * wz
    cs = wz2.sum(0)
    cb = (bz @ wz).astype(np.float32)

    wzA = np.zeros((CZ, 17), np.float16)
    wzA[:, :H] = wz2
    wzA[:, 16] = 1.0
    csI = np.zeros((128, H, 128), np.float16)
    for h in range(H):
        for p in range(128):
            csI[p, h, p] = np.float16(-cs[h])

    def btile(v):
        return np.ascontiguousarray(v.reshape(KA, 128).T.astype(np.float32))

    common = {
        "wq": np.asarray(inputs["wq"], np.float16),
        "wk": np.asarray(inputs["wk"], np.float16),
        "wv": np.asarray(inputs["wv"], np.float16),
        "wg": np.asarray(inputs["wg"], np.float16),
        "wo": np.asarray(inputs["wo"], np.float16),
        "adaln_g_w": np.asarray(inputs["adaln_g_w"], np.float16),
        "adaln_s_w": np.asarray(inputs["adaln_s_w"], np.float16),
        "w_last": np.asarray(inputs["w_last"], np.float16),
        "adaln_g_b": btile(np.asarray(inputs["adaln_g_b"], np.float32)),
        "bo_b": btile(np.asarray(inputs["bo"], np.float32)),
        "b_last_b": btile(np.asarray(inputs["b_last"], np.float32)),
        "bg_full": np.asarray(inputs["bg"], np.float32),
        "wzA17": wzA, "csI": csI, "cb": cb,
    }

    in_maps = []
    for c in range(NCORE):
        lo = c * ROWS - HALO
        hi = c * ROWS + ROWS + HALO
        a_h = np.zeros((R, CA), np.float16)
        s_h = np.zeros((R, CS), np.float16)
        g0, g1 = max(lo, 0), min(hi, N)
        a_h[g0 - lo : g1 - lo] = a[g0:g1]
        s_h[g0 - lo : g1 - lo] = s[g0:g1]
        z_c = z[c * BPC : (c + 1) * BPC]  # [16, NQ, NK, CZ]
        # zT5[g, n, cz, q*NK + k] = z_c[4g+n, q, k, cz]
        zT5 = np.ascontiguousarray(
            z_c.reshape(NGRP, 4, NQ, NK, CZ)
            .transpose(0, 1, 4, 2, 3)
            .reshape(NGRP, 4, CZ, NQ * NK)
            .astype(np.float16)
        )
        nglob = c * BPC + np.arange(BPC)
        idx = nglob[:, None] * NQ + np.arange(NK)[None, :] - OFF
        mask = np.where((idx >= 0) & (idx < N), 0.0, -30000.0).astype(np.float32)
        # maskT[k, g, n*NQ + q] = mask[4g+n, k]  (broadcast over q)
        maskT = np.ascontiguousarray(
            np.repeat(
                mask.reshape(NGRP, 4, NK).transpose(2, 0, 1)[:, :, :, None],
                NQ, axis=3,
            ).reshape(NK, NGRP, 128).astype(np.float16)
        )
        m = dict(common)
        m.update({"zT5": zT5, "a_h": a_h, "s_h": s_h, "maskT": maskT})
        in_maps.append(m)
    return in_maps


_NC_CACHE = {}


def kernel(**inputs):
    if "nc" not in _NC_CACHE:
        _NC_CACHE["nc"] = build_core_kernel()
    nc = _NC_CACHE["nc"]
    in_maps = host_prep(inputs)
    res = bass_utils.run_bass_kernel_spmd(
        nc, in_maps, core_ids=list(range(NCORE)),
        trace=bool(int(os.environ.get("KTRACE", "0"))),
    )
    kernel.last_results = res
    outs = [np.asarray(res.results[c]["outT"]).T for c in range(NCORE)]
    return np.ascontiguousarray(np.concatenate(outs, 0)[None]).astype(np.float32)


# revision 32
# speedup vs baseline: 1.0455x; 1.0455x over previous
"""AttentionPairBias Trainium2 Bass kernel — 8-core SPMD, block-sharded.

Sharding: 128 attention blocks -> 16 blocks (512 query rows) per core, with a
64-row halo on a/s so k/v windows need no cross-core exchange.

z path runs z-STATIONARY: per (group, block, query) the [cz, 128k] z tile is
loaded as PE weights and a 17-col wz matrix (16 heads + ones column for the
mean) streams through, so the pair-bias lands directly on k-partitions — no
[18 x N] evictions and no bias transposes. A second 1-col matmul over z^2
yields the variance. Scores are computed TRANSPOSED (k x q) so the exp output
is already the AV stationary; the softmax denominator rides the AV matmul as
a ones-column appended to each v window. LN stats ride the matmuls; big
projections run fp16 with fp32 accumulation.
"""
import math
import os
import sys
from contextlib import ExitStack

import numpy as np

sys.path.insert(0, "/opt/trn_rl_repo")
sys.path.insert(0, "/opt/trn_rl_repo/concourse")

import concourse.bass as bass
import concourse.mybir as mybir
import concourse.tile as tile
from concourse import bacc, bass_utils
from concourse.masks import make_identity

B, N, CA, CS, CZ, H = 1, 4096, 768, 384, 128, 16
NQ, NK = 32, 128
DH = CA // H            # 48
NB = N // NQ            # 128
OFF = (NK - NQ) // 2    # 48
NCORE = 8
BPC = NB // NCORE       # 16 blocks per core
ROWS = BPC * NQ         # 512 own rows
HALO = 64
R = ROWS + 2 * HALO     # 640 rows incl. halo
NGRP = 4                # 4-block groups per core
EPS = 1e-5
ISCALE = 1.0 / math.sqrt(DH)
KA = CA // 128          # 6
KS = CS // 128          # 3
NT = R // 128           # 5
VW = DH + 1             # 49: v window cols per head incl. ones column

FP32 = mybir.dt.float32
FP16 = mybir.dt.float16
AF = mybir.ActivationFunctionType
ALU = mybir.AluOpType


def bcast_ap(dram, parts, n):
    """DRAM [n] -> AP [[0,parts],[1,n]] (partition broadcast)."""
    a = dram[:]
    return bass.AP(tensor=a.tensor, offset=a.offset, ap=[[0, parts], [1, n]])


def build_core_kernel():
    nc = bacc.Bacc(None, target_bir_lowering=False)

    d_zT5 = nc.dram_tensor("zT5", [NGRP, 4, CZ, NQ * NK], FP16, kind="ExternalInput")
    d_a = nc.dram_tensor("a_h", [R, CA], FP16, kind="ExternalInput")
    d_s = nc.dram_tensor("s_h", [R, CS], FP16, kind="ExternalInput")
    d_wq = nc.dram_tensor("wq", [CA, CA], FP16, kind="ExternalInput")
    d_wk = nc.dram_tensor("wk", [CA, CA], FP16, kind="ExternalInput")
    d_wv = nc.dram_tensor("wv", [CA, CA], FP16, kind="ExternalInput")
    d_wg = nc.dram_tensor("wg", [CA, CA], FP16, kind="ExternalInput")
    d_wo = nc.dram_tensor("wo", [CA, CA], FP16, kind="ExternalInput")
    d_agw = nc.dram_tensor("adaln_g_w", [CS, CA], FP16, kind="ExternalInput")
    d_asw = nc.dram_tensor("adaln_s_w", [CS, CA], FP16, kind="ExternalInput")
    d_wl = nc.dram_tensor("w_last", [CS, CA], FP16, kind="ExternalInput")
    d_agb = nc.dram_tensor("adaln_g_b", [128, KA], FP32, kind="ExternalInput")
    d_bo = nc.dram_tensor("bo_b", [128, KA], FP32, kind="ExternalInput")
    d_bl = nc.dram_tensor("b_last_b", [128, KA], FP32, kind="ExternalInput")
    d_bgf = nc.dram_tensor("bg_full", [CA], FP32, kind="ExternalInput")
    d_wzA = nc.dram_tensor("wzA17", [CZ, 17], FP16, kind="ExternalInput")
    d_csI = nc.dram_tensor("csI", [128, H, 128], FP16, kind="ExternalInput")
    d_mask = nc.dram_tensor("maskT", [128, NGRP, 128], FP16, kind="ExternalInput")
    d_cb = nc.dram_tensor("cb", [H], FP32, kind="ExternalInput")
    d_out = nc.dram_tensor("outT", [CA, ROWS], FP32, kind="ExternalOutput")

    with tile.TileContext(nc) as tc, ExitStack() as ctx:
        const = ctx.enter_context(tc.tile_pool(name="const", bufs=1))
        pers = ctx.enter_context(tc.tile_pool(name="pers", bufs=1))
        ln_p = ctx.enter_context(tc.tile_pool(name="ln", bufs=2))
        at_p = ctx.enter_context(tc.tile_pool(name="attn", bufs=4))
        psA = ctx.enter_context(tc.tile_pool(name="psA", bufs=4, space="PSUM"))
        psB = ctx.enter_context(tc.tile_pool(name="psB", bufs=3, space="PSUM"))
        psZ = ctx.enter_context(tc.tile_pool(name="psZ", bufs=1, space="PSUM"))

        def pA(shape):
            return psA.tile(shape, FP32, tag="A", name="pA")

        def pB(shape, dt=FP32):
            return psB.tile(shape, dt, tag="B", name="pB")

        # ---------- constants ----------
        I16 = const.tile([128, 128], FP16, tag="I16")
        make_identity(nc, I16)
        csI = const.tile([128, H, 128], FP16, tag="csI")
        nc.gpsimd.dma_start(out=csI, in_=d_csI[:])
        maskT = const.tile([128, NGRP, 128], FP16, tag="maskT")
        nc.gpsimd.dma_start(out=maskT, in_=d_mask[:])
        wzA = const.tile([CZ, 17], FP16, tag="wzA")
        nc.gpsimd.dma_start(out=wzA, in_=d_wzA[:])
        agb = const.tile([128, KA], FP32, tag="agb")
        nc.gpsimd.dma_start(out=agb, in_=d_agb[:])
        bob = const.tile([128, KA], FP32, tag="bob")
        nc.gpsimd.dma_start(out=bob, in_=d_bo[:])
        blb = const.tile([128, KA], FP32, tag="blb")
        nc.gpsimd.dma_start(out=blb, in_=d_bl[:])
        bg_bc = const.tile([128, CA], FP32, tag="bg_bc")
        nc.gpsimd.dma_start(out=bg_bc, in_=bcast_ap(d_bgf, 128, CA))
        cb_sb = const.tile([128, H], FP32, tag="cb_sb")
        nc.gpsimd.dma_start(out=cb_sb, in_=bcast_ap(d_cb, 128, H))
        epsb = const.tile([128, 1], FP32, tag="epsb")
        nc.vector.memset(epsb, EPS)

        def load_w(pool, dram, kt, name):
            t = pool.tile([128, kt, dram.shape[1]], FP16, tag=name)
            nc.gpsimd.dma_start(out=t, in_=dram[:].rearrange("(k p) c -> p k c", p=128))
            return t

        # ---------- persistent activations ----------
        sT = pers.tile([128, KS, ROWS], FP16, tag="sT")
        qT_hd = pers.tile([128, H // 2, ROWS], FP16, tag="qT_hd")
        kT_hd = pers.tile([128, H // 2, R], FP16, tag="kT_hd")
        vw = pers.tile([128, BPC, H * VW], FP16, tag="vw")
        g_rm = pers.tile([128, NGRP, CA], FP16, tag="g_rm")
        go_rm = pers.tile([128, NGRP, CA], FP16, tag="go_rm")
        goT = pers.tile([128, KA, ROWS], FP16, tag="goT")
        thall_a = pers.tile([128, NGRP, H, 128], FP16, tag="thall_a")
        gs_all = pers.tile([128, KA, ROWS], FP16, tag="gs_all")
        mur_a = pers.tile([128, NGRP, 128], FP16, tag="mur_a")
        mid_cm = tc.tile_pool(name="mid", bufs=1)
        mid = mid_cm.__enter__()
        a2T = mid.tile([128, KA, R], FP16, tag="a2T")

        # ========== Phase 0: z-stationary bias pipeline (emitted interleaved
        # with phase 1 — depends only on the z input, so it overlaps the LN
        # and projection phases; DMAs ride the idle Sync queue) ==========
        zb_cm = [
            tc.tile_pool(name="zp", bufs=2),
            tc.tile_pool(name="bp", bufs=1),
            tc.tile_pool(name="sp", bufs=1),
        ]
        zp, bp, sp = [cm.__enter__() for cm in zb_cm]
        _zstate = {}

        def z_chunk(g, n):
            braw = _zstate.get(g)
            if braw is None:
                braw = bp.tile([128, 4, NQ, 18], FP16, tag="braw")
                _zstate[g] = braw
            zt = zp.tile([128, NQ * NK], FP16, tag="zt")
            nc.sync.dma_start(out=zt, in_=d_zT5[g, n])
            idx = 4 * g + n
            for qh in range(2):
                ztsq = zp.tile([128, NQ * NK // 2], FP16, tag="ztsq")
                zh = zt[:, qh * 2048 : (qh + 1) * 2048]
                hi = (2 * idx + qh) % 8
                if hi < 4:
                    nc.vector.tensor_mul(ztsq, zh, zh)
                elif hi < 6:
                    nc.scalar.square(out=ztsq, in_=zh)
                else:
                    nc.gpsimd.tensor_mul(ztsq, zh, zh)
                PZ = psZ.tile([128, 16, 18], FP32, tag="Z", name="pZ")
                for qq in range(16):
                    q = qh * 16 + qq
                    nc.tensor.matmul(
                        PZ[:, qq, 0:17],
                        zt[:, q * NK : (q + 1) * NK],
                        wzA, start=True, stop=True,
                    )
                    nc.tensor.matmul(
                        PZ[:, qq, 17:18],
                        ztsq[:, qq * NK : (qq + 1) * NK],
                        wzA[:, 16:17], start=True, stop=True,
                    )
                dst = braw[:, n, qh * 16 : (qh + 1) * 16, :]
                if qh == 0:
                    nc.scalar.activation(out=dst, in_=PZ, func=AF.Copy)
                else:
                    nc.vector.tensor_copy(out=dst, in_=PZ)

        def z_stats(g):
            braw = _zstate.pop(g)
            st = sp.tile([128, 2, 4, NQ], FP32, tag="st")
            nc.scalar.mul(out=st[:, 0], in_=braw[:, :, :, 16], mul=1.0 / CZ)
            nc.scalar.mul(out=st[:, 1], in_=braw[:, :, :, 17], mul=1.0 / CZ)
            var = sp.tile([128, 4, NQ], FP32, tag="var")
            nc.vector.tensor_mul(var, st[:, 0], st[:, 0])
            nc.vector.tensor_sub(var, st[:, 1], var)
            sd = sp.tile([128, 4, NQ], FP32, tag="sd")
            nc.scalar.activation(out=sd, in_=var, func=AF.Sqrt, bias=epsb)
            rstd = sp.tile([128, 4, NQ], FP32, tag="rstd")
            nc.vector.reciprocal(out=rstd, in_=sd)
            murv = mur_a[:, g, :].rearrange("p (n q) -> p n q", n=4)
            nc.vector.tensor_mul(murv, st[:, 0], rstd)
            in0 = braw[:, :, :, 0:16].transpose([0, 3, 1, 2])
            in1 = rstd[:].unsqueeze(1).broadcast_to([128, H, 4, NQ])
            outv = thall_a[:, g].rearrange("p h (n q) -> p h n q", n=4)
            nc.vector.tensor_mul(outv, in0, in1)
            if g in (0, NGRP - 1):
                mbc = maskT[:, g, :].unsqueeze(1).broadcast_to([128, H, 128])
                nc.vector.tensor_add(thall_a[:, g], thall_a[:, g], mbc)

        z_units = []
        for g in range(NGRP):
            for n in range(4):
                z_units.append(lambda g=g, n=n: z_chunk(g, n))
            z_units.append(lambda g=g: z_stats(g))
        _zit = iter(z_units)

        def z_step(k=1):
            for _ in range(k):
                u = next(_zit, None)
                if u is not None:
                    u()

        # ================= Phase 1: LN + adaln -> a2T =================
        with tc.tile_pool(name="ph1", bufs=1) as ph1:
            w_ag = load_w(ph1, d_agw, KS, "w_ag")
            w_as = load_w(ph1, d_asw, KS, "w_as")
            s_lnT = ph1.tile([128, KS, R], FP16, tag="s_lnT")
            a_lnT = ph1.tile([128, KA, R], FP16, tag="a_lnT")

            def layernorm_rowtile(dst_T, src_dram, width, t, kt):
                x = ln_p.tile([128, width], FP16, tag=f"ln_x{width}")
                nc.gpsimd.dma_start(out=x, in_=src_dram[t * 128 : (t + 1) * 128, :])
                sub = math.gcd(512, width)
                nsub = width // sub
                st = ln_p.tile([128, nsub, 6], FP32, tag=f"ln_st{width}")
                for j in range(nsub):
                    nc.vector.bn_stats(out=st[:, j, :], in_=x[:, j * sub : (j + 1) * sub])
                mv = ln_p.tile([128, 2], FP32, tag=f"ln_mv{width}")
                nc.vector.bn_aggr(out=mv, in_=st)
                sd = ln_p.tile([128, 1], FP32, tag=f"ln_sd{width}")
                nc.scalar.activation(out=sd, in_=mv[:, 1:2], func=AF.Sqrt, bias=epsb)
                nc.vector.reciprocal(out=sd, in_=sd)
                x16 = ln_p.tile([128, width], FP16, tag=f"ln_x16{width}")
                nc.vector.tensor_scalar(
                    out=x16, in0=x, scalar1=mv[:, 0:1], scalar2=sd,
                    op0=ALU.subtract, op1=ALU.mult,
                )
                for k in range(kt):
                    pt = pB([128, 128], FP16)
                    nc.tensor.transpose(pt, x16[:, k * 128 : (k + 1) * 128], I16)
                    if k % 2 == 0:
                        nc.vector.tensor_copy(
                            out=dst_T[:, k, t * 128 : (t + 1) * 128], in_=pt
                        )
                    else:
                        nc.scalar.activation(
                            out=dst_T[:, k, t * 128 : (t + 1) * 128], in_=pt,
                            func=AF.Copy,
                        )

            for t in range(NT):
                layernorm_rowtile(s_lnT, d_s, CS, t, KS)
                z_step()
                layernorm_rowtile(a_lnT, d_a, CA, t, KA)
                z_step()

            # raw s transposed (own rows) for the final gate
            for t in range(ROWS // 128):
                x = ln_p.tile([128, CS], FP16, tag="s_raw")
                nc.gpsimd.dma_start(
                    out=x, in_=d_s[HALO + t * 128 : HALO + (t + 1) * 128, :]
                )
                for k in range(KS):
                    pt = pB([128, 128], FP16)
                    nc.tensor.transpose(pt, x[:, k * 128 : (k + 1) * 128], I16)
                    nc.vector.tensor_copy(out=sT[:, k, t * 128 : (t + 1) * 128], in_=pt)
                z_step()

            RC = 320
            for co in range(KA):
                for rc in range(R // RC):
                    z_step()
                    rs = slice(rc * RC, (rc + 1) * RC)
                    pg = pA([128, RC])
                    pv = pA([128, RC])
                    for k in range(KS):
                        nc.tensor.matmul(
                            pg, (w_ag[:, k, co * 128 : (co + 1) * 128]),
                            (s_lnT[:, k, rs]), start=(k == 0), stop=(k == KS - 1),
                        )
                    for k in range(KS):
                        nc.tensor.matmul(
                            pv, (w_as[:, k, co * 128 : (co + 1) * 128]),
                            (s_lnT[:, k, rs]), start=(k == 0), stop=(k == KS - 1),
                        )
                    gate = ln_p.tile([128, RC], FP32, tag="gate")
                    nc.scalar.activation(
                        out=gate, in_=pg, func=AF.Sigmoid, bias=agb[:, co : co + 1]
                    )
                    nc.vector.tensor_mul(gate, gate, a_lnT[:, co, rs])
                    nc.vector.tensor_add(a2T[:, co, rs], gate, pv)

        # drain any remaining z-bias work, then free its pools
        z_step(32)
        for cm in reversed(zb_cm):
            cm.__exit__(None, None, None)

        # ================= Phase 2a: q/k projections + head remap =================
        with tc.tile_pool(name="ph2a", bufs=1) as ph2a:
            w_q = load_w(ph2a, d_wq, KA, "w_q")
            w_k = load_w(ph2a, d_wk, KA, "w_k")
            qT_ct = ph2a.tile([128, KA, ROWS], FP16, tag="qT_ct")
            kT_ct = ph2a.tile([128, KA, R], FP16, tag="kT_ct")
            for co in range(KA):
                for rc in range(2):
                    rs = slice(HALO + rc * 256, HALO + (rc + 1) * 256)
                    pq = pA([128, 256])
                    for k in range(KA):
                        nc.tensor.matmul(
                            pq, (w_q[:, k, co * 128 : (co + 1) * 128]),
                            (a2T[:, k, rs]), start=(k == 0), stop=(k == KA - 1),
                        )
                    nc.vector.tensor_scalar(
                        out=qT_ct[:, co, rc * 256 : (rc + 1) * 256], in0=pq,
                        scalar1=ISCALE, scalar2=None, op0=ALU.mult,
                    )
                for rc in range(2):
                    rs = slice(rc * 320, (rc + 1) * 320)
                    pk = pA([128, 320])
                    for k in range(KA):
                        nc.tensor.matmul(
                            pk, (w_k[:, k, co * 128 : (co + 1) * 128]),
                            (a2T[:, k, rs]), start=(k == 0), stop=(k == KA - 1),
                        )
                    nc.scalar.activation(out=kT_ct[:, co, rs], in_=pk, func=AF.Copy)
            # head-aligned remap (2 heads per tile, 64-padded)
            for h in range(H):
                hp, o64 = h // 2, (h % 2) * 64
                c0 = h * DH
                while c0 < (h + 1) * DH:
                    ct, cp = c0 // 128, c0 % 128
                    ln_ = min((h + 1) * DH - c0, 128 - cp)
                    dsl = slice(o64 + (c0 - h * DH), o64 + (c0 - h * DH) + ln_)
                    nc.sync.dma_start(
                        out=qT_hd[dsl, hp, :], in_=qT_ct[cp : cp + ln_, ct, :]
                    )
                    nc.sync.dma_start(
                        out=kT_hd[dsl, hp, :], in_=kT_ct[cp : cp + ln_, ct, :]
                    )
                    c0 += ln_

        # ================= Phase 2b: v (row-major) + g =================
        with tc.tile_pool(name="ph2b", bufs=1) as ph2b:
            w_v = load_w(ph2b, d_wv, KA, "w_v")
            w_g = load_w(ph2b, d_wg, KA, "w_g")
            v_rm = ph2b.tile([128, NT, H * VW], FP16, tag="v_rm")
            ones_v = v_rm[:].rearrange("p t (h c) -> p t h c", h=H)[:, :, :, DH : DH + 1]
            nc.vector.memset(ones_v, 1.0)

            def vw_window(n):
                wlo = 16 + 32 * n
                t0, p0 = wlo // 128, wlo % 128
                n0 = 128 - p0
                nc.gpsimd.dma_start(out=vw[0:n0, n, :], in_=v_rm[p0 : p0 + n0, t0, :])
                if n0 < 128:
                    nc.gpsimd.dma_start(
                        out=vw[n0:128, n, :], in_=v_rm[0 : 128 - n0, t0 + 1, :]
                    )

            for rt in range(NT):
                for c2 in range(2):
                    pv = pA([128, 384])
                    for k in range(KA):
                        nc.tensor.matmul(
                            pv, (a2T[:, k, rt * 128 : (rt + 1) * 128]),
                            (w_v[:, k, c2 * 384 : (c2 + 1) * 384]),
                            start=(k == 0), stop=(k == KA - 1),
                        )
                    dstv = v_rm[:, rt, :].rearrange("p (h c) -> p h c", h=H)[
                        :, c2 * 8 : (c2 + 1) * 8, 0:DH
                    ]
                    nc.vector.tensor_copy(out=dstv, in_=pv)
                if rt >= 1:
                    for n in range(4 * (rt - 1), 4 * rt):
                        vw_window(n)
            for rt in range(ROWS // 128):
                for c2 in range(2):
                    pg = pA([128, 384])
                    for k in range(KA):
                        nc.tensor.matmul(
                            pg, (a2T[:, k, HALO + rt * 128 : HALO + (rt + 1) * 128]),
                            (w_g[:, k, c2 * 384 : (c2 + 1) * 384]),
                            start=(k == 0), stop=(k == KA - 1),
                        )
                    nc.vector.tensor_add(pg, pg, bg_bc[:, c2 * 384 : (c2 + 1) * 384])
                    nc.scalar.activation(
                        out=g_rm[:, rt, c2 * 384 : (c2 + 1) * 384], in_=pg,
                        func=AF.Sigmoid,
                    )
            # remaining key/value windows
            for n in range(4 * (NT - 1), BPC):
                vw_window(n)

        mid_cm.__exit__(None, None, None)
        # s-conditioned output gate, precomputed so the attention phase's
        # scalar queue runs Exp only (no activation-table thrash)
        with tc.tile_pool(name="ph7a", bufs=1) as ph7a:
            w_l = load_w(ph7a, d_wl, KS, "w_l")
            for co in range(KA):
                pl = pA([128, ROWS])
                for k in range(KS):
                    nc.tensor.matmul(
                        pl, (w_l[:, k, co * 128 : (co + 1) * 128]),
                        (sT[:, k, :]), start=(k == 0), stop=(k == KS - 1),
                    )
                nc.scalar.activation(
                    out=gs_all[:, co, :], in_=pl, func=AF.Sigmoid,
                    bias=blb[:, co : co + 1],
                )
        # ============ Phase 3-5: transposed attention (bias precomputed) ============
        # software-pipelined by 2: scores/exp for (g,h) are emitted two
        # iterations before the AV/normalize stage that consumes them, so the
        # PE never stalls head-of-line on the scalar exp.
        def att_scores(g, h):
            hp, o64 = h // 2, (h % 2) * 64
            S = pB([128, NK])
            nc.tensor.matmul(S, I16, thall_a[:, g, h, :], start=True, stop=False)
            nc.tensor.matmul(S, csI[:, h, :], mur_a[:, g, :], start=False, stop=False)
            for nn in range(4):
                n = g * 4 + nn
                wlo = 16 + 32 * n
                nc.tensor.matmul(
                    S[:, nn * NQ : (nn + 1) * NQ],
                    kT_hd[o64 : o64 + DH, hp, wlo : wlo + NK],
                    qT_hd[o64 : o64 + DH, hp, n * NQ : (n + 1) * NQ],
                    start=False, stop=(nn == 3), skip_group_check=True,
                )
            p16 = at_p.tile([128, NK], FP16, tag="p16")
            nc.scalar.activation(out=p16, in_=S, func=AF.Exp, bias=cb_sb[:, h : h + 1])
            return p16

        def att_out(g, h, p16):
            OV = pB([128, VW])
            for nn in range(4):
                n = g * 4 + nn
                nc.tensor.matmul(
                    OV[nn * NQ : (nn + 1) * NQ, :],
                    p16[:, nn * NQ : (nn + 1) * NQ],
                    vw[:, n, h * VW : (h + 1) * VW],
                    start=True, stop=True,
                    tile_position=(0, nn * NQ),
                )
            rec = at_p.tile([128, 1], FP32, tag="rec")
            nc.vector.reciprocal(out=rec, in_=OV[:, DH : DH + 1])
            nc.vector.scalar_tensor_tensor(
                out=go_rm[:, g, h * DH : (h + 1) * DH], in0=OV[:, 0:DH],
                scalar=rec, in1=g_rm[:, g, h * DH : (h + 1) * DH],
                op0=ALU.mult, op1=ALU.mult,
            )

        pipe = []
        for g in range(NGRP):
            for h in range(H):
                pipe.append((g, h, att_scores(g, h)))
                if len(pipe) > 2:
                    pg_, ph_, pp_ = pipe.pop(0)
                    att_out(pg_, ph_, pp_)
            # (g*o) transpose for this group (lags the pipeline by <=3 heads)
            if g > 0:
                for k in range(KA):
                    pt = pB([128, 128], FP16)
                    nc.tensor.transpose(
                        pt, go_rm[:, g - 1, k * 128 : (k + 1) * 128], I16
                    )
                    nc.vector.tensor_copy(
                        out=goT[:, k, (g - 1) * 128 : g * 128], in_=pt
                    )
        for pg_, ph_, pp_ in pipe:
            att_out(pg_, ph_, pp_)
        for k in range(KA):
            pt = pB([128, 128], FP16)
            nc.tensor.transpose(
                pt, go_rm[:, NGRP - 1, k * 128 : (k + 1) * 128], I16
            )
            nc.vector.tensor_copy(
                out=goT[:, k, (NGRP - 1) * 128 : NGRP * 128], in_=pt
            )

        # ================= Phase 7: output projection + gates =================
        with tc.tile_pool(name="ph7", bufs=1) as ph7:
            w_o = load_w(ph7, d_wo, KA, "w_o")
            for gh in range(2):
                rs = slice(gh * 256, (gh + 1) * 256)
                for co in range(KA):
                    po = pA([128, 256])
                    for k in range(KA):
                        nc.tensor.matmul(
                            po, (w_o[:, k, co * 128 : (co + 1) * 128]),
                            (goT[:, k, rs]), start=(k == 0), stop=(k == KA - 1),
                        )
                    o2 = at_p.tile([128, 256], FP32, tag="o2")
                    nc.vector.tensor_scalar(
                        out=o2, in0=po, scalar1=bob[:, co : co + 1], scalar2=None,
                        op0=ALU.add,
                    )
                    fin = at_p.tile([128, 256], FP32, tag="fin")
                    nc.vector.tensor_mul(fin, gs_all[:, co, rs], o2)
                    nc.gpsimd.dma_start(out=d_out[co * 128 : (co + 1) * 128, rs], in_=fin)

    nc.compile()
    return nc


def host_prep(inputs):
    a = np.ascontiguousarray(np.asarray(inputs["a"], np.float32)[0])
    s = np.ascontiguousarray(np.asarray(inputs["s"], np.float32)[0])
    z = np.asarray(inputs["z"], np.float32)[0]
    gz = np.asarray(inputs["gz"], np.float32)
    bz = np.asarray(inputs["bz"], np.float32)
    wz = np.asarray(inputs["wz"], np.float32)
    wz2 = gz[:, None] * wz
    cs = wz2.sum(0)
    cb = (bz @ wz).astype(np.float32)

    wzA = np.zeros((CZ, 17), np.float16)
    wzA[:, :H] = wz2
    wzA[:, 16] = 1.0
    csI = np.zeros((128, H, 128), np.float16)
    for h in range(H):
        for p in range(128):
            csI[p, h, p] = np.float16(-cs[h])

    def btile(v):
        return np.ascontiguousarray(v.reshape(KA, 128).T.astype(np.float32))

    common = {
        "wq": np.asarray(inputs["wq"], np.float16),
        "wk": np.asarray(inputs["wk"], np.float16),
        "wv": np.asarray(inputs["wv"], np.float16),
        "wg": np.asarray(inputs["wg"], np.float16),
        "wo": np.asarray(inputs["wo"], np.float16),
        "adaln_g_w": np.asarray(inputs["adaln_g_w"], np.float16),
        "adaln_s_w": np.asarray(inputs["adaln_s_w"], np.float16),
        "w_last": np.asarray(inputs["w_last"], np.float16),
        "adaln_g_b": btile(np.asarray(inputs["adaln_g_b"], np.float32)),
        "bo_b": btile(np.asarray(inputs["bo"], np.float32)),
        "b_last_b": btile(np.asarray(inputs["b_last"], np.float32)),
        "bg_full": np.asarray(inputs["bg"], np.float32),
        "wzA17": wzA, "csI": csI, "cb": cb,
    }

    in_maps = []
    for c in range(NCORE):
        lo = c * ROWS - HALO
        hi = c * ROWS + ROWS + HALO
        a_h = np.zeros((R, CA), np.float16)
        s_h = np.zeros((R, CS), np.float16)
        g0, g1 = max(lo, 0), min(hi, N)
        a_h[g0 - lo : g1 - lo] = a[g0:g1]
        s_h[g0 - lo : g1 - lo] = s[g0:g1]
        z_c = z[c * BPC : (c + 1) * BPC]  # [16, NQ, NK, CZ]
        # zT5[g, n, cz, q*NK + k] = z_c[4g+n, q, k, cz]
        zT5 = np.ascontiguousarray(
            z_c.reshape(NGRP, 4, NQ, NK, CZ)
            .transpose(0, 1, 4, 2, 3)
            .reshape(NGRP, 4, CZ, NQ * NK)
            .astype(np.float16)
        )
        nglob = c * BPC + np.arange(BPC)
        idx = nglob[:, None] * NQ + np.arange(NK)[None, :] - OFF
        mask = np.where((idx >= 0) & (idx < N), 0.0, -30000.0).astype(np.float32)
        # maskT[k, g, n*NQ + q] = mask[4g+n, k]  (broadcast over q)
        maskT = np.ascontiguousarray(
            np.repeat(
                mask.reshape(NGRP, 4, NK).transpose(2, 0, 1)[:, :, :, None],
                NQ, axis=3,
            ).reshape(NK, NGRP, 128).astype(np.float16)
        )
        m = dict(common)
        m.update({"zT5": zT5, "a_h": a_h, "s_h": s_h, "maskT": maskT})
        in_maps.append(m)
    return in_maps


_NC_CACHE = {}


def kernel(**inputs):
    if "nc" not in _NC_CACHE:
        _NC_CACHE["nc"] = build_core_kernel()
    nc = _NC_CACHE["nc"]
    in_maps = host_prep(inputs)
    res = bass_utils.run_bass_kernel_spmd(
        nc, in_maps, core_ids=list(range(NCORE)),
        trace=bool(int(os.environ.get("KTRACE", "0"))),
    )
    kernel.last_results = res
    outs = [np.asarray(res.results[c]["outT"]).T for c in range(NCORE)]
    return np.ascontiguousarray(np.concatenate(outs, 0)[None]).astype(np.float32)
